# revision 2
# baseline (speedup 1.0000x reference)
"""Trainium2 Bass kernel for fused multi-head attention (B=4, N=2048, D=384, h=8, dh=48).

Sharding: 32 (batch, head) pairs across 8 cores -> core c handles batch c//2 and
heads [4*(c%2), 4*(c%2)+4). Each core computes a *partial* output projection
(its 4 heads' contribution to out @ Wproj); the host sums the two partials per
batch and adds bproj.

Per-core algorithm (everything in "transposed" layout so no PE transposes are
needed):
  xT   [384, 2048]  = x^T                          (transposed on host, bf16)
  QT   [256, 2048]  = (Wq_pad)^T @ xT  (4 heads padded dh 48->64, pair-packed)
  KT   same
  V'   [2048, 4*64] = x @ Wv_pad, with a ones-column per head at col h*64+32
                      (gpsimd memset) -> the PV matmul also accumulates the
                      softmax denominator Z for free.
  attention, software-pipelined at 512-query granularity: per (pair, q-half,
  head, key-chunk kc, j-half):
      simT[k, q512] = KT_h^T @ QT_h   (PSUM [128,512] = 1 bank, 4-deep ring;
                                       sims run 2 kc-steps ahead of PV)
      E = exp(simT)  j=0 -> ACT engine (exact exp, psum->sbuf bf16)
                     j=1 -> DVE via Schraudolph bit-trick: i16 = s*A + B
                            bitcast to bf16 ~= exp(s) (+-4% per weight, which
                            softmax-averages to ~2e-3 on the final output;
                            scores ~N(0,1) so no max subtraction needed)
      acc[o:o+64, q] += V'_kc^T @ E   (PSUM accumulate; row o+32 = Z)
  OT_h = acc[o:o+64] * (1/Z)          (DVE approx-reciprocal + gpsimd partition
                                       broadcast + DVE multiply), bf16
  y    [2048, 384]  = sum_h OT_h^T @ Wproj_h   (partial; f32 out; first half
                      interleaved into the third attention block)
"""

import os

os.environ.pop("JAX_PLATFORMS", None)  # the bass PJRT path needs the axon platform

import numpy as np
import ml_dtypes

import concourse.mybir as mybir
import concourse.tile as tile
from concourse import bacc
from concourse.bass_utils import run_bass_kernel_spmd

BF16 = ml_dtypes.bfloat16

# problem shapes (hardcoded per contract)
B, N, D = 4, 2048, 384
H, DH = 8, 48
SCALE = DH**-0.5
N_CORES = 8
HP = 4  # heads per core
DHP = 64  # padded head dim
P = 128
NKC = N // P  # 16 key-row chunks
ZOFF = 32  # partition offset of the fused softmax-denominator (Z) row within a
# head's 64-row block: engines need 32-aligned partition starts, so the ones
# column sits at col 32 of each head's V' block; v-dims occupy cols
# [0,32) and [33,49), the rest are zero. Wproj rows are laid out to match,
# with zeros at the Z/pad rows.

# Schraudolph exp for the DVE half: exp(s) ~= bitcast_bf16(int16(s*A + B)).
# A = 128/ln2; B = 127*128 + delta with delta centering the piecewise-linear
# 2^frac ~ 1+frac error (worst case ~+-4%, validated end-to-end offline).
SCHR_A = 128.0 / 0.6931471805599453
SCHR_B = 16248.87

LAST_EXEC_NS = None
_CACHE = {}


def _build_bass():
    f32 = mybir.dt.float32
    bf16 = mybir.dt.bfloat16
    i16 = mybir.dt.int16
    EXP = mybir.ActivationFunctionType.Exp
    MULT = mybir.AluOpType.mult
    ADD = mybir.AluOpType.add

    nc = bacc.Bacc("TRN2", target_bir_lowering=False, debug=False, num_devices=N_CORES)
    xbT = nc.dram_tensor("xbT", [D, N], bf16, kind="ExternalInput").ap()
    wq = nc.dram_tensor("wq", [D, HP * DHP], bf16, kind="ExternalInput").ap()
    wk = nc.dram_tensor("wk", [D, HP * DHP], bf16, kind="ExternalInput").ap()
    wv = nc.dram_tensor("wv", [D, HP * DHP], bf16, kind="ExternalInput").ap()
    wpj = nc.dram_tensor("wpj", [2, P, D], bf16, kind="ExternalInput").ap()
    y = nc.dram_tensor("y", [N, D], f32, kind="ExternalOutput").ap()

    with tile.TileContext(nc) as tc:
        with (
            tc.tile_pool(name="const", bufs=1) as cpool,
            tc.tile_pool(name="epool", bufs=8) as epool,
            tc.tile_pool(name="rpool", bufs=3) as rpool,
            tc.tile_pool(name="ysb", bufs=6) as ypool,
            tc.tile_pool(name="simps", bufs=4, space="PSUM") as simps,
            tc.tile_pool(name="accps", bufs=2, space="PSUM") as accps,
        ):
            # ---- load weights / x ----
            # Early-DMA queue plan: weights on the gpsimd SWDGE queue (K first
            # - the first matmul groups need it), xT halves alternating across
            # the two HWDGE queues so the first qkv groups unblock early.
            wq_sb, wk_sb, wv_sb = [], [], []
            for name, srct, dst in (("wk", wk, wk_sb), ("wq", wq, wq_sb), ("wv", wv, wv_sb)):
                for i in range(3):
                    t = cpool.tile([P, HP * DHP], bf16, name=f"{name}{i}", tag=f"{name}{i}")
                    nc.gpsimd.dma_start(out=t[:], in_=srct[i * P : (i + 1) * P, :])
                    dst.append(t)
            xT = [cpool.tile([P, N], bf16, name=f"xT{i}", tag=f"xT{i}") for i in range(3)]
            for hf in range(2):  # halves so the first qkv groups unblock early
                for i in range(3):
                    eng = nc.sync if (i + hf) % 2 == 0 else nc.scalar
                    eng.dma_start(
                        out=xT[i][:, hf * (N // 2) : (hf + 1) * (N // 2)],
                        in_=xbT[i * P : (i + 1) * P, hf * (N // 2) : (hf + 1) * (N // 2)],
                    )
            wpj_sb = []
            for p in range(2):
                t = cpool.tile([P, D], bf16, name=f"wpj{p}", tag=f"wpj{p}")
                nc.gpsimd.dma_start(out=t[:], in_=wpj[p])
                wpj_sb.append(t)

            # ---- QKV projection ----
            QT = [cpool.tile([P, N], bf16, name=f"QT{p}", tag=f"QT{p}") for p in range(2)]
            KT = [cpool.tile([P, N], bf16, name=f"KT{p}", tag=f"KT{p}") for p in range(2)]

            def qk_pair(p):
                for gi, (w_sb, dstl) in enumerate(((wk_sb, KT), (wq_sb, QT))):
                    for j in range(4):
                        pool = simps if (gi * 4 + j) % 2 == 0 else accps
                        ps = pool.tile([P, 512], f32, name="qkvps", tag="sim" if pool is simps else "acc")
                        for dk in range(3):
                            nc.tensor.matmul(
                                ps[:],
                                lhsT=w_sb[dk][:, p * P : (p + 1) * P],
                                rhs=xT[dk][:, j * 512 : (j + 1) * 512],
                                start=(dk == 0),
                                stop=(dk == 2),
                            )
                        if j % 2 == 0:
                            nc.vector.tensor_copy(dstl[p][:, j * 512 : (j + 1) * 512], ps[:])
                        else:
                            nc.scalar.copy(dstl[p][:, j * 512 : (j + 1) * 512], ps[:])

            qk_pair(0)
            qk_pair(1)
            V = [cpool.tile([P, HP * DHP], bf16, name=f"V{i}", tag=f"V{i}") for i in range(NKC)]

            def v_chunks(i0, i1):
                for i in range(i0, i1):
                    pool = simps if i % 2 == 0 else accps
                    ps = pool.tile([P, HP * DHP], f32, name="vps", tag="sim" if pool is simps else "acc")
                    for dk in range(3):
                        nc.tensor.matmul(
                            ps[:],
                            lhsT=xT[dk][:, i * P : (i + 1) * P],
                            rhs=wv_sb[dk][:],
                            start=(dk == 0),
                            stop=(dk == 2),
                        )
                    t = V[i]
                    if i % 2 == 0:
                        nc.vector.tensor_copy(t[:], ps[:])
                    else:
                        nc.scalar.copy(t[:], ps[:])
                    # ones (Z) column of each head block, at col h*64+ZOFF
                    zcols = t[:].rearrange("p (h c) -> p h c", c=DHP)[:, :, ZOFF : ZOFF + 1]
                    nc.gpsimd.memset(zcols, 1.0)

            v_chunks(0, NKC)

            # ---- attention: software-pipelined over 128 (p,qh,hh,kc) steps ----
            OT = [cpool.tile([P, N], bf16, name=f"OT{p}", tag=f"OT{p}") for p in range(2)]
            BLOCKS = ((0, 0), (1, 0), (0, 1), (1, 1))
            steps = [
                (p, qh, hh, kc)
                for (p, qh) in BLOCKS
                for hh in range(2)
                for kc in range(NKC)
            ]
            nsteps = len(steps)  # 128
            LOOK = 2  # kc-steps the sims run ahead of the PVs

            accs = {}
            es = {}

            def emit_sim(si, j):
                p, qh, hh, kc = steps[si]
                o = hh * DHP
                sp = simps.tile([P, 512], f32, name="sim", tag="sim")
                nc.tensor.matmul(
                    sp[:],
                    lhsT=KT[p][o : o + DHP, kc * P : (kc + 1) * P],
                    rhs=QT[p][o : o + DHP, qh * 1024 + j * 512 : qh * 1024 + (j + 1) * 512],
                    start=True,
                    stop=True,
                )
                e = epool.tile([P, 512], bf16, name="E", tag="E")
                if j == 0:
                    nc.scalar.activation(e[:], sp[:], EXP)
                else:
                    nc.vector.tensor_scalar(e[:].bitcast(i16), sp[:], SCHR_A, SCHR_B, MULT, ADD)
                es[(si, j)] = e

            def emit_pv(si, j):
                p, qh, hh, kc = steps[si]
                o = hh * DHP
                h = p * 2 + hh
                if si % 32 == 0 and j == 0:
                    accs[si // 32] = accps.tile([P, 1024], f32, name="acc", tag="acc")
                acc = accs[si // 32]
                e = es.pop((si, j))
                nc.tensor.matmul(
                    acc[o : o + DHP, j * 512 : (j + 1) * 512],
                    lhsT=V[kc][:, h * DHP : (h + 1) * DHP],
                    rhs=e[:],
                    start=(kc == 0),
                    stop=(kc == NKC - 1),
                )

            def norm_chunk(bi, hh, s, w):
                # OT[o:o+64, chunk] = acc[o:o+64, chunk] / Z  (Z at row o+ZOFF)
                p, qh = BLOCKS[bi]
                o = hh * DHP
                acc = accs[bi]
                zrow = rpool.tile([1, w], f32, name="zrow", tag="zrow")
                nc.vector.tensor_copy(zrow[:], acc[o + ZOFF : o + ZOFF + 1, s * w : (s + 1) * w])
                r = rpool.tile([1, w], f32, name="r", tag="r")
                nc.vector.reciprocal_approx_fast(r[:], zrow[:])
                R = rpool.tile([DHP, w], f32, name="R", tag="R")
                nc.gpsimd.partition_broadcast(R[:], r[:], channels=DHP)
                nc.vector.tensor_mul(
                    OT[p][o : o + DHP, qh * 1024 + s * w : qh * 1024 + (s + 1) * w],
                    acc[o : o + DHP, s * w : (s + 1) * w],
                    R[:],
                )

            # ---- output projection (partial: this core's 4 heads) ----
            def emit_proj(mc):
                pool = simps if mc % 2 == 0 else accps
                yp = pool.tile([P, D], f32, name="yp", tag="sim" if pool is simps else "acc")
                for p in range(2):  # K=128 covers both heads of the pair
                    nc.tensor.matmul(
                        yp[:],
                        lhsT=OT[p][:, mc * P : (mc + 1) * P],
                        rhs=wpj_sb[p][:],
                        start=(p == 0),
                        stop=(p == 1),
                    )
                ys = ypool.tile([P, D], f32, name="ys", tag="ys")
                if mc % 2 == 0:
                    nc.vector.tensor_copy(ys[:], yp[:])
                    nc.sync.dma_start(out=y[mc * P : (mc + 1) * P, :], in_=ys[:])
                else:
                    nc.scalar.copy(ys[:], yp[:])
                    nc.gpsimd.dma_start(out=y[mc * P : (mc + 1) * P, :], in_=ys[:])

            proj_pending = []  # mc's ready to emit, drained 1/iteration

            for s in range(LOOK):
                emit_sim(s, 0)
                emit_sim(s, 1)
            for s in range(nsteps):
                if s + LOOK < nsteps:
                    emit_sim(s + LOOK, 0)
                    emit_sim(s + LOOK, 1)
                emit_pv(s, 0)
                emit_pv(s, 1)
                if proj_pending:
                    emit_proj(proj_pending.pop(0))
                if s % 32 == 31:
                    bi = s // 32
                    if bi < 3:
                        for hh in range(2):
                            norm_chunk(bi, hh, 0, 1024)
                        accs.pop(bi)
                        if bi == 1:
                            # OT[0]+OT[1] for qh=0 done -> first 8 proj chunks
                            proj_pending.extend(range(8))
                    else:
                        # last block: finer chunks so the tail projs start sooner
                        for s_ in range(2):
                            for hh in range(2):
                                norm_chunk(bi, hh, s_, 512)
                            for mc in range(8 + 4 * s_, 12 + 4 * s_):
                                emit_proj(mc)
                        accs.pop(bi)

    nc.compile()
    return nc


def _prep_core_inputs(x, Wqkv, Wproj, core):
    b, hg = core // 2, core % 2
    heads = [hg * HP + i for i in range(HP)]
    xbT = np.ascontiguousarray(x[b].astype(BF16).T)
    wq = np.zeros((D, HP * DHP), np.float32)
    wk = np.zeros((D, HP * DHP), np.float32)
    wv = np.zeros((D, HP * DHP), np.float32)
    wpj = np.zeros((2, P, D), np.float32)
    for i, h in enumerate(heads):
        wq[:, i * DHP : i * DHP + DH] = Wqkv[:, h * DH : (h + 1) * DH] * SCALE
        wk[:, i * DHP : i * DHP + DH] = Wqkv[:, H * DH + h * DH : H * DH + (h + 1) * DH]
        wv_h = Wqkv[:, 2 * H * DH + h * DH : 2 * H * DH + (h + 1) * DH]
        wpj_h = Wproj[h * DH : (h + 1) * DH, :]
        # v-dims at cols [0,ZOFF) and [ZOFF+1, DH+1); ones (Z) column at ZOFF
        wv[:, i * DHP : i * DHP + ZOFF] = wv_h[:, :ZOFF]
        wv[:, i * DHP + ZOFF + 1 : i * DHP + DH + 1] = wv_h[:, ZOFF:]
        o = (i % 2) * DHP
        wpj[i // 2, o : o + ZOFF, :] = wpj_h[:ZOFF, :]
        wpj[i // 2, o + ZOFF + 1 : o + DH + 1, :] = wpj_h[ZOFF:, :]
    return {
        "xbT": xbT,
        "wq": wq.astype(BF16),
        "wk": wk.astype(BF16),
        "wv": wv.astype(BF16),
        "wpj": wpj.astype(BF16),
    }


def kernel(x, Wqkv, Wproj, bproj):
    global LAST_EXEC_NS
    if "nc" not in _CACHE:
        _CACHE["nc"] = _build_bass()
    nc = _CACHE["nc"]
    in_maps = [_prep_core_inputs(x, Wqkv, Wproj, c) for c in range(N_CORES)]
    try:
        res = run_bass_kernel_spmd(nc, in_maps, core_ids=list(range(N_CORES)))
    except Exception:
        res = run_bass_kernel_spmd(nc, in_maps, core_ids=list(range(N_CORES)))
    LAST_EXEC_NS = res.exec_time_ns
    out = np.empty((B, N, D), np.float32)
    for b in range(B):
        out[b] = res.results[2 * b]["y"] + res.results[2 * b + 1]["y"]
    out += bproj.astype(np.float32)[None, None, :]
    return out


# revision 6
# speedup vs baseline: 1.0787x; 1.0787x over previous
"""Trainium2 Bass kernel for fused multi-head attention (B=4, N=2048, D=384, h=8, dh=48).

Sharding: 32 (batch, head) pairs across 8 cores -> core c handles batch c//2 and
heads [4*(c%2), 4*(c%2)+4). Each core computes a *partial* output projection
(its 4 heads' contribution to out @ Wproj); the host sums the two partials per
batch and adds bproj.

Per-core algorithm (everything in "transposed" layout so no PE transposes are
needed):
  xT   [384, 2048]  = x^T                          (transposed on host, bf16)
  QT   [256, 2048]  = (Wq_pad)^T @ xT  (4 heads padded dh 48->64, pair-packed)
  KT   same
  V'   [2048, 4*64] = x @ Wv_pad, with a ones-column per head at col h*64+32
                      (gpsimd memset) -> the PV matmul also accumulates the
                      softmax denominator Z for free.
  attention, software-pipelined at 512-query granularity: per (pair, q-half,
  head, key-chunk kc, j-half):
      simT[k, q512] = KT_h^T @ QT_h   (PSUM [128,512] = 1 bank, 4-deep ring;
                                       sims run 2 kc-steps ahead of PV)
      E = exp(simT)  j=0 -> ACT engine (exact exp, psum->sbuf bf16)
                     j=1 -> DVE via Schraudolph bit-trick: i16 = s*A + B
                            bitcast to bf16 ~= exp(s) (+-4% per weight, which
                            softmax-averages to ~2e-3 on the final output;
                            scores ~N(0,1) so no max subtraction needed)
      acc[o:o+64, q] += V'_kc^T @ E   (PSUM accumulate; row o+32 = Z)
  OT_h = acc[o:o+64] * (1/Z)          (DVE approx-reciprocal + gpsimd partition
                                       broadcast + DVE multiply), bf16
  y    [2048, 384]  = sum_h OT_h^T @ Wproj_h   (partial; f32 out; first half
                      interleaved into the third attention block)
"""

import os

os.environ.pop("JAX_PLATFORMS", None)  # the bass PJRT path needs the axon platform

import numpy as np
import ml_dtypes

import concourse.mybir as mybir
import concourse.tile as tile
from concourse import bacc
from concourse.bass_utils import run_bass_kernel_spmd

BF16 = ml_dtypes.bfloat16

# problem shapes (hardcoded per contract)
B, N, D = 4, 2048, 384
H, DH = 8, 48
SCALE = DH**-0.5
N_CORES = 8
HP = 4  # heads per core
DHP = 64  # padded head dim
P = 128
NKC = N // P  # 16 key-row chunks
ZOFF = 32  # partition offset of the fused softmax-denominator (Z) row within a
# head's 64-row block: engines need 32-aligned partition starts, so the ones
# column sits at col 32 of each head's V' block; v-dims occupy cols
# [0,32) and [33,49), the rest are zero. Wproj rows are laid out to match,
# with zeros at the Z/pad rows.

# Schraudolph exp for the DVE half: exp(s) ~= bitcast_bf16(int16(s*A + B)).
# A = 128/ln2; B = 127*128 + delta with delta centering the piecewise-linear
# 2^frac ~ 1+frac error (worst case ~+-4%, validated end-to-end offline).
SCHR_A = 128.0 / 0.6931471805599453
SCHR_B = 16248.87

LAST_EXEC_NS = None
_CACHE = {}


def _build_bass():
    f32 = mybir.dt.float32
    bf16 = mybir.dt.bfloat16
    i16 = mybir.dt.int16
    EXP = mybir.ActivationFunctionType.Exp
    MULT = mybir.AluOpType.mult
    ADD = mybir.AluOpType.add

    nc = bacc.Bacc("TRN2", target_bir_lowering=False, debug=False, num_devices=N_CORES)
    xbT = nc.dram_tensor("xbT", [D, N], bf16, kind="ExternalInput").ap()
    wq = nc.dram_tensor("wq", [D, HP * DHP], bf16, kind="ExternalInput").ap()
    wk = nc.dram_tensor("wk", [D, HP * DHP], bf16, kind="ExternalInput").ap()
    wv = nc.dram_tensor("wv", [D, HP * DHP], bf16, kind="ExternalInput").ap()
    wpj = nc.dram_tensor("wpj", [2, P, D], bf16, kind="ExternalInput").ap()
    y = nc.dram_tensor("y", [N, D], f32, kind="ExternalOutput").ap()

    with tile.TileContext(nc) as tc:
        with (
            tc.tile_pool(name="const", bufs=1) as cpool,
            tc.tile_pool(name="epool", bufs=8) as epool,
            tc.tile_pool(name="rpool", bufs=3) as rpool,
            tc.tile_pool(name="ysb", bufs=6) as ypool,
            tc.tile_pool(name="simps", bufs=5, space="PSUM") as simps,
            tc.tile_pool(name="accps", bufs=3, space="PSUM") as accps,
        ):
            # ---- load weights / x ----
            # Early-DMA queue plan: weights on the gpsimd SWDGE queue (K first
            # - the first matmul groups need it), xT halves alternating across
            # the two HWDGE queues so the first qkv groups unblock early.
            wq_sb, wk_sb, wv_sb = [], [], []
            for name, srct, dst in (("wk", wk, wk_sb), ("wq", wq, wq_sb), ("wv", wv, wv_sb)):
                for i in range(3):
                    t = cpool.tile([P, HP * DHP], bf16, name=f"{name}{i}", tag=f"{name}{i}")
                    nc.gpsimd.dma_start(out=t[:], in_=srct[i * P : (i + 1) * P, :])
                    dst.append(t)
            xT = [cpool.tile([P, N], bf16, name=f"xT{i}", tag=f"xT{i}") for i in range(3)]
            for hf in range(2):  # halves so the first qkv groups unblock early
                for i in range(3):
                    eng = nc.sync if (i + hf) % 2 == 0 else nc.scalar
                    eng.dma_start(
                        out=xT[i][:, hf * (N // 2) : (hf + 1) * (N // 2)],
                        in_=xbT[i * P : (i + 1) * P, hf * (N // 2) : (hf + 1) * (N // 2)],
                    )
            wpj_sb = []
            for p in range(2):
                t = cpool.tile([P, D], bf16, name=f"wpj{p}", tag=f"wpj{p}")
                nc.gpsimd.dma_start(out=t[:], in_=wpj[p])
                wpj_sb.append(t)

            # ---- QKV projection ----
            QT = [cpool.tile([P, N], bf16, name=f"QT{p}", tag=f"QT{p}") for p in range(2)]
            KT = [cpool.tile([P, N], bf16, name=f"KT{p}", tag=f"KT{p}") for p in range(2)]

            def qk_pair(p):
                for gi, (w_sb, dstl) in enumerate(((wk_sb, KT), (wq_sb, QT))):
                    for j in range(4):
                        pool = simps if (gi * 4 + j) % 2 == 0 else accps
                        ps = pool.tile([P, 512], f32, name="qkvps", tag="sim" if pool is simps else "acc")
                        for dk in range(3):
                            nc.tensor.matmul(
                                ps[:],
                                lhsT=w_sb[dk][:, p * P : (p + 1) * P],
                                rhs=xT[dk][:, j * 512 : (j + 1) * 512],
                                start=(dk == 0),
                                stop=(dk == 2),
                            )
                        if j % 2 == 0:
                            nc.vector.tensor_copy(dstl[p][:, j * 512 : (j + 1) * 512], ps[:])
                        else:
                            nc.scalar.copy(dstl[p][:, j * 512 : (j + 1) * 512], ps[:])

            qk_pair(0)
            qk_pair(1)
            V = [cpool.tile([P, HP * DHP], bf16, name=f"V{i}", tag=f"V{i}") for i in range(NKC)]

            def v_chunks(i0, i1):
                for i in range(i0, i1):
                    pool = simps if i % 2 == 0 else accps
                    ps = pool.tile([P, HP * DHP], f32, name="vps", tag="sim" if pool is simps else "acc")
                    for dk in range(3):
                        nc.tensor.matmul(
                            ps[:],
                            lhsT=xT[dk][:, i * P : (i + 1) * P],
                            rhs=wv_sb[dk][:],
                            start=(dk == 0),
                            stop=(dk == 2),
                        )
                    t = V[i]
                    if i % 2 == 0:
                        nc.vector.tensor_copy(t[:], ps[:])
                    else:
                        nc.scalar.copy(t[:], ps[:])
                    # ones (Z) column of each head block, at col h*64+ZOFF
                    zcols = t[:].rearrange("p (h c) -> p h c", c=DHP)[:, :, ZOFF : ZOFF + 1]
                    nc.gpsimd.memset(zcols, 1.0)

            v_chunks(0, NKC)

            # ---- attention: software-pipelined over 128 (p,qh,hh,kc) steps ----
            OT = [cpool.tile([P, N], bf16, name=f"OT{p}", tag=f"OT{p}") for p in range(2)]
            BLOCKS = ((0, 0), (1, 0), (0, 1), (1, 1))
            steps = [
                (p, qh, hh, kc)
                for (p, qh) in BLOCKS
                for hh in range(2)
                for kc in range(NKC)
            ]
            nsteps = len(steps)  # 128
            LOOK = 2  # kc-steps the sims run ahead of the PVs

            accs = {}
            es = {}

            def emit_sim(si, j):
                p, qh, hh, kc = steps[si]
                o = hh * DHP
                sp = simps.tile([P, 512], f32, name="sim", tag="sim")
                nc.tensor.matmul(
                    sp[:],
                    lhsT=KT[p][o : o + DHP, kc * P : (kc + 1) * P],
                    rhs=QT[p][o : o + DHP, qh * 1024 + j * 512 : qh * 1024 + (j + 1) * 512],
                    start=True,
                    stop=True,
                )
                e = epool.tile([P, 512], bf16, name="E", tag="E")
                if j == 0:
                    nc.scalar.activation(e[:], sp[:], EXP)
                else:
                    nc.vector.tensor_scalar(e[:].bitcast(i16), sp[:], SCHR_A, SCHR_B, MULT, ADD)
                es[(si, j)] = e

            def emit_pv(si, j):
                p, qh, hh, kc = steps[si]
                o = hh * DHP
                h = p * 2 + hh
                if si % 32 == 0:
                    # one 1-bank accumulator half per j (3-deep ring across blocks)
                    accs[(si // 32, j)] = accps.tile([P, 512], f32, name="acc", tag="acc")
                acc = accs[(si // 32, j)]
                e = es.pop((si, j))
                nc.tensor.matmul(
                    acc[o : o + DHP, :],
                    lhsT=V[kc][:, h * DHP : (h + 1) * DHP],
                    rhs=e[:],
                    start=(kc == 0),
                    stop=(kc == NKC - 1),
                )

            def norm_chunk(bi, hh, s_):
                # OT[o:o+64, chunk] = acc[o:o+64, chunk] / Z  (Z at row o+ZOFF)
                p, qh = BLOCKS[bi]
                o = hh * DHP
                acc = accs[(bi, s_)]
                zrow = rpool.tile([1, 512], f32, name="zrow", tag="zrow")
                nc.vector.tensor_copy(zrow[:], acc[o + ZOFF : o + ZOFF + 1, :])
                r = rpool.tile([1, 512], f32, name="r", tag="r")
                nc.vector.reciprocal_approx_fast(r[:], zrow[:])
                R = rpool.tile([DHP, 512], f32, name="R", tag="R")
                nc.gpsimd.partition_broadcast(R[:], r[:], channels=DHP)
                nc.vector.tensor_mul(
                    OT[p][o : o + DHP, qh * 1024 + s_ * 512 : qh * 1024 + (s_ + 1) * 512],
                    acc[o : o + DHP, :],
                    R[:],
                )

            # ---- output projection (partial: this core's 4 heads) ----
            def emit_proj(mc):
                # yp always from the sim ring: the acc ring holds live block
                # accumulators, so parking projs there would chain Tensor
                # behind a future block's normalize.
                yp = simps.tile([P, D], f32, name="yp", tag="sim")
                for p in range(2):  # K=128 covers both heads of the pair
                    nc.tensor.matmul(
                        yp[:],
                        lhsT=OT[p][:, mc * P : (mc + 1) * P],
                        rhs=wpj_sb[p][:],
                        start=(p == 0),
                        stop=(p == 1),
                    )
                ys = ypool.tile([P, D], f32, name="ys", tag="ys")
                if mc % 2 == 0:
                    nc.vector.tensor_copy(ys[:], yp[:])
                    nc.sync.dma_start(out=y[mc * P : (mc + 1) * P, :], in_=ys[:])
                else:
                    nc.scalar.copy(ys[:], yp[:])
                    nc.gpsimd.dma_start(out=y[mc * P : (mc + 1) * P, :], in_=ys[:])

            proj_pending = []  # mc's ready to emit, drained 1 per 4 iterations

            for s in range(LOOK):
                emit_sim(s, 0)
                emit_sim(s, 1)
            for s in range(nsteps):
                if s + LOOK < nsteps:
                    emit_sim(s + LOOK, 0)
                    emit_sim(s + LOOK, 1)
                emit_pv(s, 0)
                emit_pv(s, 1)
                if proj_pending and s % 4 == 0:
                    emit_proj(proj_pending.pop(0))
                if s % 32 == 31:
                    bi = s // 32
                    if bi < 3:
                        for hh in range(2):
                            for s_ in range(2):
                                norm_chunk(bi, hh, s_)
                        accs.pop((bi, 0))
                        accs.pop((bi, 1))
                        if bi == 1:
                            # OT[0]+OT[1] for qh=0 done -> first 8 proj chunks
                            proj_pending.extend(range(8))
                    else:
                        # last block: s_-major so the tail projs start sooner
                        for s_ in range(2):
                            for hh in range(2):
                                norm_chunk(bi, hh, s_)
                            for mc in range(8 + 4 * s_, 12 + 4 * s_):
                                emit_proj(mc)
                        accs.pop((bi, 0))
                        accs.pop((bi, 1))

    nc.compile()
    return nc


def _prep_core_inputs(x, Wqkv, Wproj, core):
    b, hg = core // 2, core % 2
    heads = [hg * HP + i for i in range(HP)]
    xbT = np.ascontiguousarray(x[b].astype(BF16).T)
    wq = np.zeros((D, HP * DHP), np.float32)
    wk = np.zeros((D, HP * DHP), np.float32)
    wv = np.zeros((D, HP * DHP), np.float32)
    wpj = np.zeros((2, P, D), np.float32)
    for i, h in enumerate(heads):
        wq[:, i * DHP : i * DHP + DH] = Wqkv[:, h * DH : (h + 1) * DH] * SCALE
        wk[:, i * DHP : i * DHP + DH] = Wqkv[:, H * DH + h * DH : H * DH + (h + 1) * DH]
        wv_h = Wqkv[:, 2 * H * DH + h * DH : 2 * H * DH + (h + 1) * DH]
        wpj_h = Wproj[h * DH : (h + 1) * DH, :]
        # v-dims at cols [0,ZOFF) and [ZOFF+1, DH+1); ones (Z) column at ZOFF
        wv[:, i * DHP : i * DHP + ZOFF] = wv_h[:, :ZOFF]
        wv[:, i * DHP + ZOFF + 1 : i * DHP + DH + 1] = wv_h[:, ZOFF:]
        o = (i % 2) * DHP
        wpj[i // 2, o : o + ZOFF, :] = wpj_h[:ZOFF, :]
        wpj[i // 2, o + ZOFF + 1 : o + DH + 1, :] = wpj_h[ZOFF:, :]
    return {
        "xbT": xbT,
        "wq": wq.astype(BF16),
        "wk": wk.astype(BF16),
        "wv": wv.astype(BF16),
        "wpj": wpj.astype(BF16),
    }


def kernel(x, Wqkv, Wproj, bproj):
    global LAST_EXEC_NS
    if "nc" not in _CACHE:
        _CACHE["nc"] = _build_bass()
    nc = _CACHE["nc"]
    in_maps = [_prep_core_inputs(x, Wqkv, Wproj, c) for c in range(N_CORES)]
    try:
        res = run_bass_kernel_spmd(nc, in_maps, core_ids=list(range(N_CORES)))
    except Exception:
        res = run_bass_kernel_spmd(nc, in_maps, core_ids=list(range(N_CORES)))
    LAST_EXEC_NS = res.exec_time_ns
    out = np.empty((B, N, D), np.float32)
    for b in range(B):
        out[b] = res.results[2 * b]["y"] + res.results[2 * b + 1]["y"]
    out += bproj.astype(np.float32)[None, None, :]
    return out


# revision 10
# speedup vs baseline: 1.2607x; 1.1687x over previous
"""Trainium2 Bass kernel for fused multi-head attention (B=4, N=2048, D=384, h=8, dh=48).

Sharding: 32 (batch, head) pairs across 8 cores -> core c handles batch c//2 and
heads [4*(c%2), 4*(c%2)+4). Each core computes a *partial* output projection
(its 4 heads' contribution to out @ Wproj); the host sums the two partials per
batch and adds bproj.

Per-core algorithm (everything in "transposed" layout so no PE transposes are
needed):
  xT   [384, 2048]  = x^T                          (transposed on host, bf16)
  QT   [256, 2048]  = (Wq_pad)^T @ xT  (4 heads padded dh 48->64, pair-packed)
  KT   same
  V'   [2048, 4*64] = x @ Wv_pad, with a ones-column per head at col h*64+32
                      (gpsimd memset) -> the PV matmul also accumulates the
                      softmax denominator Z for free.
  attention, software-pipelined at 512-query granularity: per (pair, q-half,
  head, key-chunk kc, j-half):
      simT[k, q512] = KT_h^T @ QT_h   (PSUM [128,512] = 1 bank, 4-deep ring;
                                       sims run 2 kc-steps ahead of PV)
      E = exp(simT)  j=0 -> ACT engine (exact exp, psum->sbuf bf16)
                     j=1 -> DVE via Schraudolph bit-trick: i16 = s*A + B
                            bitcast to bf16 ~= exp(s) (+-4% per weight, which
                            softmax-averages to ~2e-3 on the final output;
                            scores ~N(0,1) so no max subtraction needed)
      acc[o:o+64, q] += V'_kc^T @ E   (PSUM accumulate; row o+32 = Z)
  OT_h = acc[o:o+64] * (1/Z)          (DVE approx-reciprocal + gpsimd partition
                                       broadcast + DVE multiply), bf16
  y    [2048, 384]  = sum_h OT_h^T @ Wproj_h   (partial; f32 out; first half
                      interleaved into the third attention block)
"""

import os

os.environ.pop("JAX_PLATFORMS", None)  # the bass PJRT path needs the axon platform

import numpy as np
import ml_dtypes

import concourse.mybir as mybir
import concourse.tile as tile
from concourse import bacc
from concourse.bass_utils import run_bass_kernel_spmd

BF16 = ml_dtypes.bfloat16

# problem shapes (hardcoded per contract)
B, N, D = 4, 2048, 384
H, DH = 8, 48
SCALE = DH**-0.5
N_CORES = 8
HP = 4  # heads per core
DHP = 64  # padded head dim
P = 128
NKC = N // P  # 16 key-row chunks
ZOFF = 32  # partition offset of the fused softmax-denominator (Z) row within a
# head's 64-row block: engines need 32-aligned partition starts, so the ones
# column sits at col 32 of each head's V' block; v-dims occupy cols
# [0,32) and [33,49), the rest are zero. Wproj rows are laid out to match,
# with zeros at the Z/pad rows.

# Schraudolph exp for the DVE half: exp(s) ~= bitcast_bf16(int16(s*A + B)).
# A = 128/ln2; B = 127*128 + delta with delta centering the piecewise-linear
# 2^frac ~ 1+frac error (worst case ~+-4%, validated end-to-end offline).
SCHR_A = 128.0 / 0.6931471805599453
SCHR_B = 16248.87

LAST_EXEC_NS = None
_CACHE = {}


def _build_bass():
    f32 = mybir.dt.float32
    bf16 = mybir.dt.bfloat16
    i16 = mybir.dt.int16
    EXP = mybir.ActivationFunctionType.Exp
    MULT = mybir.AluOpType.mult
    ADD = mybir.AluOpType.add

    nc = bacc.Bacc("TRN2", target_bir_lowering=False, debug=False, num_devices=N_CORES)
    xbT = nc.dram_tensor("xbT", [D, N], bf16, kind="ExternalInput").ap()
    wq = nc.dram_tensor("wq", [D, HP * DHP], bf16, kind="ExternalInput").ap()
    wk = nc.dram_tensor("wk", [D, HP * DHP], bf16, kind="ExternalInput").ap()
    wv = nc.dram_tensor("wv", [D, HP * DHP], bf16, kind="ExternalInput").ap()
    wpj = nc.dram_tensor("wpj", [2, P, D], bf16, kind="ExternalInput").ap()
    y = nc.dram_tensor("y", [N, D], f32, kind="ExternalOutput").ap()

    with tile.TileContext(nc) as tc:
        with (
            tc.tile_pool(name="const", bufs=1) as cpool,
            tc.tile_pool(name="epool", bufs=8) as epool,
            tc.tile_pool(name="rpool", bufs=3) as rpool,
            tc.tile_pool(name="ysb", bufs=6) as ypool,
            tc.tile_pool(name="simps", bufs=5, space="PSUM") as simps,
            tc.tile_pool(name="accps", bufs=3, space="PSUM") as accps,
        ):
            # ---- load weights / x ----
            # Early-DMA queue plan: weights on the gpsimd SWDGE queue (K first
            # - the first matmul groups need it), xT halves alternating across
            # the two HWDGE queues so the first qkv groups unblock early.
            wq_sb, wk_sb, wv_sb = [], [], []
            for name, srct, dst in (("wk", wk, wk_sb), ("wq", wq, wq_sb), ("wv", wv, wv_sb)):
                for i in range(3):
                    t = cpool.tile([P, HP * DHP], bf16, name=f"{name}{i}", tag=f"{name}{i}")
                    nc.gpsimd.dma_start(out=t[:], in_=srct[i * P : (i + 1) * P, :])
                    dst.append(t)
            xT = [cpool.tile([P, N], bf16, name=f"xT{i}", tag=f"xT{i}") for i in range(3)]
            for hf in range(2):  # halves so the first qkv groups unblock early
                for i in range(3):
                    eng = nc.sync if (i + hf) % 2 == 0 else nc.scalar
                    eng.dma_start(
                        out=xT[i][:, hf * (N // 2) : (hf + 1) * (N // 2)],
                        in_=xbT[i * P : (i + 1) * P, hf * (N // 2) : (hf + 1) * (N // 2)],
                    )
            wpj_sb = []
            for p in range(2):
                t = cpool.tile([P, D], bf16, name=f"wpj{p}", tag=f"wpj{p}")
                nc.gpsimd.dma_start(out=t[:], in_=wpj[p])
                wpj_sb.append(t)

            # ---- QKV projection ----
            QT = [cpool.tile([P, N], bf16, name=f"QT{p}", tag=f"QT{p}") for p in range(2)]
            KT = [cpool.tile([P, N], bf16, name=f"KT{p}", tag=f"KT{p}") for p in range(2)]

            def qk_pair(p):
                for gi, (w_sb, dstl) in enumerate(((wk_sb, KT), (wq_sb, QT))):
                    for j in range(4):
                        pool = simps if (gi * 4 + j) % 2 == 0 else accps
                        ps = pool.tile([P, 512], f32, name="qkvps", tag="sim" if pool is simps else "acc")
                        for dk in range(3):
                            nc.tensor.matmul(
                                ps[:],
                                lhsT=w_sb[dk][:, p * P : (p + 1) * P],
                                rhs=xT[dk][:, j * 512 : (j + 1) * 512],
                                start=(dk == 0),
                                stop=(dk == 2),
                            )
                        if j % 2 == 0:
                            nc.vector.tensor_copy(dstl[p][:, j * 512 : (j + 1) * 512], ps[:])
                        else:
                            nc.scalar.copy(dstl[p][:, j * 512 : (j + 1) * 512], ps[:])

            qk_pair(0)
            qk_pair(1)
            V = [cpool.tile([P, HP * DHP], bf16, name=f"V{i}", tag=f"V{i}") for i in range(NKC)]

            def v_chunks(i0, i1):
                for i in range(i0, i1):
                    pool = simps if i % 2 == 0 else accps
                    ps = pool.tile([P, HP * DHP], f32, name="vps", tag="sim" if pool is simps else "acc")
                    for dk in range(3):
                        nc.tensor.matmul(
                            ps[:],
                            lhsT=xT[dk][:, i * P : (i + 1) * P],
                            rhs=wv_sb[dk][:],
                            start=(dk == 0),
                            stop=(dk == 2),
                        )
                    t = V[i]
                    if i % 2 == 0:
                        nc.vector.tensor_copy(t[:], ps[:])
                    else:
                        nc.scalar.copy(t[:], ps[:])
                    # ones (Z) column of each head block, at col h*64+ZOFF
                    zcols = t[:].rearrange("p (h c) -> p h c", c=DHP)[:, :, ZOFF : ZOFF + 1]
                    nc.gpsimd.memset(zcols, 1.0)

            v_chunks(0, NKC)

            # ---- attention: software-pipelined over 128 (p,qh,hh,kc) steps ----
            OT = [cpool.tile([P, N], bf16, name=f"OT{p}", tag=f"OT{p}") for p in range(2)]
            BLOCKS = ((0, 0), (1, 0), (0, 1), (1, 1))
            steps = [
                (p, qh, hh, kc)
                for (p, qh) in BLOCKS
                for hh in range(2)
                for kc in range(NKC)
            ]
            nsteps = len(steps)  # 128
            LOOK = 2  # kc-steps the sims run ahead of the PVs

            accs = {}
            es = {}

            def emit_sim(si, j):
                p, qh, hh, kc = steps[si]
                o = hh * DHP
                sp = simps.tile([P, 512], f32, name="sim", tag="sim")
                nc.tensor.matmul(
                    sp[:],
                    lhsT=KT[p][o : o + DHP, kc * P : (kc + 1) * P],
                    rhs=QT[p][o : o + DHP, qh * 1024 + j * 512 : qh * 1024 + (j + 1) * 512],
                    start=True,
                    stop=True,
                )
                e = epool.tile([P, 512], bf16, name="E", tag="E")
                # 5/8 of tiles on ACT (exact exp), 3/8 on DVE (Schraudolph):
                # leaves the DVE slack to absorb the normalize bursts.
                if j == 0 or si % 4 == 3:
                    nc.scalar.activation(e[:], sp[:], EXP)
                else:
                    nc.vector.tensor_scalar(e[:].bitcast(i16), sp[:], SCHR_A, SCHR_B, MULT, ADD)
                es[(si, j)] = e

            def emit_pv(si, j):
                p, qh, hh, kc = steps[si]
                o = hh * DHP
                h = p * 2 + hh
                if si % 32 == 0:
                    # one 1-bank accumulator half per j (3-deep ring across blocks)
                    accs[(si // 32, j)] = accps.tile([P, 512], f32, name="acc", tag="acc")
                acc = accs[(si // 32, j)]
                e = es.pop((si, j))
                nc.tensor.matmul(
                    acc[o : o + DHP, :],
                    lhsT=V[kc][:, h * DHP : (h + 1) * DHP],
                    rhs=e[:],
                    start=(kc == 0),
                    stop=(kc == NKC - 1),
                )

            def norm_chunk(bi, hh, s_):
                # OT[o:o+64, chunk] = acc[o:o+64, chunk] / Z  (Z at row o+ZOFF)
                p, qh = BLOCKS[bi]
                o = hh * DHP
                acc = accs[(bi, s_)]
                zrow = rpool.tile([1, 512], f32, name="zrow", tag="zrow")
                nc.vector.tensor_copy(zrow[:], acc[o + ZOFF : o + ZOFF + 1, :])
                r = rpool.tile([1, 512], f32, name="r", tag="r")
                nc.vector.reciprocal_approx_fast(r[:], zrow[:])
                R = rpool.tile([DHP, 512], f32, name="R", tag="R")
                nc.gpsimd.partition_broadcast(R[:], r[:], channels=DHP)
                nc.vector.tensor_mul(
                    OT[p][o : o + DHP, qh * 1024 + s_ * 512 : qh * 1024 + (s_ + 1) * 512],
                    acc[o : o + DHP, :],
                    R[:],
                )

            # ---- output projection (partial: this core's 4 heads) ----
            def emit_proj(mc):
                # yp always from the sim ring: the acc ring holds live block
                # accumulators, so parking projs there would chain Tensor
                # behind a future block's normalize.
                yp = simps.tile([P, D], f32, name="yp", tag="sim")
                for p in range(2):  # K=128 covers both heads of the pair
                    nc.tensor.matmul(
                        yp[:],
                        lhsT=OT[p][:, mc * P : (mc + 1) * P],
                        rhs=wpj_sb[p][:],
                        start=(p == 0),
                        stop=(p == 1),
                    )
                ys = ypool.tile([P, D], f32, name="ys", tag="ys")
                if mc % 2 == 0:
                    nc.vector.tensor_copy(ys[:], yp[:])
                    nc.sync.dma_start(out=y[mc * P : (mc + 1) * P, :], in_=ys[:])
                else:
                    nc.scalar.copy(ys[:], yp[:])
                    nc.gpsimd.dma_start(out=y[mc * P : (mc + 1) * P, :], in_=ys[:])

            proj_pending = []  # mc's ready to emit, drained 1 per 4 iterations
            norm_pending = []  # deferred normalize chunks, drained 1 per 4 iters

            for s in range(LOOK):
                emit_sim(s, 0)
                emit_sim(s, 1)
            for s in range(nsteps):
                if s + LOOK < nsteps:
                    emit_sim(s + LOOK, 0)
                    emit_sim(s + LOOK, 1)
                emit_pv(s, 0)
                emit_pv(s, 1)
                if proj_pending and s % 4 == 0:
                    emit_proj(proj_pending.pop(0))
                if norm_pending and s % 4 == 2:
                    bi, hh, s_ = norm_pending.pop(0)
                    norm_chunk(bi, hh, s_)
                    if not any(n[0] == bi and n[2] == s_ for n in norm_pending):
                        accs.pop((bi, s_))
                if s % 32 == 31:
                    bi = s // 32
                    if bi < 3:
                        # free the j=0 accumulator now (its PSUM slot is the
                        # next ring reuse); defer the j=1 chunks into the next
                        # block so the DVE isn't swamped at the boundary
                        norm_chunk(bi, 0, 0)
                        norm_chunk(bi, 1, 0)
                        accs.pop((bi, 0))
                        norm_pending.extend([(bi, 0, 1), (bi, 1, 1)])
                        if bi == 1:
                            # OT[0]+OT[1] for qh=0 done -> first 8 proj chunks
                            proj_pending.extend(range(8))
                    else:
                        # last block: s_-major so the tail projs start sooner
                        for s_ in range(2):
                            for hh in range(2):
                                norm_chunk(bi, hh, s_)
                            for mc in range(8 + 4 * s_, 12 + 4 * s_):
                                emit_proj(mc)
                        accs.pop((bi, 0))
                        accs.pop((bi, 1))

    nc.compile()
    return nc


def _prep_core_inputs(x, Wqkv, Wproj, core):
    b, hg = core // 2, core % 2
    heads = [hg * HP + i for i in range(HP)]
    xbT = np.ascontiguousarray(x[b].astype(BF16).T)
    wq = np.zeros((D, HP * DHP), np.float32)
    wk = np.zeros((D, HP * DHP), np.float32)
    wv = np.zeros((D, HP * DHP), np.float32)
    wpj = np.zeros((2, P, D), np.float32)
    for i, h in enumerate(heads):
        wq[:, i * DHP : i * DHP + DH] = Wqkv[:, h * DH : (h + 1) * DH] * SCALE
        wk[:, i * DHP : i * DHP + DH] = Wqkv[:, H * DH + h * DH : H * DH + (h + 1) * DH]
        wv_h = Wqkv[:, 2 * H * DH + h * DH : 2 * H * DH + (h + 1) * DH]
        wpj_h = Wproj[h * DH : (h + 1) * DH, :]
        # v-dims at cols [0,ZOFF) and [ZOFF+1, DH+1); ones (Z) column at ZOFF
        wv[:, i * DHP : i * DHP + ZOFF] = wv_h[:, :ZOFF]
        wv[:, i * DHP + ZOFF + 1 : i * DHP + DH + 1] = wv_h[:, ZOFF:]
        o = (i % 2) * DHP
        wpj[i // 2, o : o + ZOFF, :] = wpj_h[:ZOFF, :]
        wpj[i // 2, o + ZOFF + 1 : o + DH + 1, :] = wpj_h[ZOFF:, :]
    return {
        "xbT": xbT,
        "wq": wq.astype(BF16),
        "wk": wk.astype(BF16),
        "wv": wv.astype(BF16),
        "wpj": wpj.astype(BF16),
    }


def kernel(x, Wqkv, Wproj, bproj):
    global LAST_EXEC_NS
    if "nc" not in _CACHE:
        _CACHE["nc"] = _build_bass()
    nc = _CACHE["nc"]
    in_maps = [_prep_core_inputs(x, Wqkv, Wproj, c) for c in range(N_CORES)]
    try:
        res = run_bass_kernel_spmd(nc, in_maps, core_ids=list(range(N_CORES)))
    except Exception:
        res = run_bass_kernel_spmd(nc, in_maps, core_ids=list(range(N_CORES)))
    LAST_EXEC_NS = res.exec_time_ns
    out = np.empty((B, N, D), np.float32)
    for b in range(B):
        out[b] = res.results[2 * b]["y"] + res.results[2 * b + 1]["y"]
    out += bproj.astype(np.float32)[None, None, :]
    return out
